# revision 1
# baseline (speedup 1.0000x reference)
"""Trainium2 Bass kernel: unnormalized single-head attention block.

Computes, for x [4, 4096, 1024] and w_q/w_k/w_v/w_o [1024, 1024] (all fp32):
    q = x @ w_q ; k = x @ w_k ; v = x @ w_v
    scores = q @ k.T            (no softmax)
    out = (scores @ v) @ w_o

Sharding: 8 NeuronCores = (4 batches) x (2 sequence halves). Each core
computes the output rows for its 2048-row half of one batch. The host passes
x.T with the core's own half first ("rotated" column order); attention sums
over s are order-independent, so all per-core tensors use that rotated
order consistently.

K projection is computed for the own half only; the peer half arrives via a
masked ReduceScatter over pair groups [[0,1],[2,3],[4,5],[6,7]]: each core
stages its K into both halves of a double buffer scaled by a host-supplied
0/1 mask (own slot zeroed), so the add-reduce-scatter delivers exactly the
peer's K into a uniform buffer on every core -- no rank-dependent addressing
in the SPMD program. V is cheaper to recompute than to exchange on this
fabric (collective transfers are ~100us for 4MB and serialize on the CC
core), so each core projects V over the full rotated sequence.

Device math is bf16 with fp32 PSUM accumulation. Layout chaining (no
on-device transposes anywhere):
    qT[e,t] = wq.T @ x.T        lhsT=wq tile,   rhs=xT
    kT[e,s] = wk.T @ x.T        lhsT=wk tile,   rhs=xT
    v[s,e]  = x @ wv            lhsT=xT tile,   rhs=wv
    sT[s,t] = k @ q.T           lhsT=kT tile,   rhs=qT
    aT[e,t] = v.T @ s           lhsT=v tile,    rhs=sT
    out[t,e]= a @ wo            lhsT=aT tile,   rhs=wo
"""

import contextlib
import ctypes
import os
import sys
import types

import numpy as np

B = 4
T = 4096
D = 1024
H = T // 2          # rows per core
P = 128             # SBUF partitions
NCORES = 8
DT = D // P         # 8 tiles along any 1024 dim
ST = T // P         # 32 tiles along the full sequence
STH = H // P        # 16 own-half s-tiles
FREE = 512          # matmul moving free dim / PSUM bank width (fp32)
SBLK = T // FREE    # 8 full-sequence blocks of 512
CH = H // FREE      # 4 t-chunks per core
GROUPS = [[0, 1], [2, 3], [4, 5], [6, 7]]

_STATE = {}
LAST_RESULTS = None


def _install_axon_ntff_shim():
    """bass_utils(trace=True) under axon imports antenv.axon_hooks, which the
    agent image lacks. Provide the documented ctypes equivalent so tracing
    works; degrades to hook=None when the .so has no profile symbols."""
    try:
        import antenv.axon_hooks  # noqa: F401
        return
    except ImportError:
        pass

    so_path = "/opt/axon/libaxon_pjrt.so"

    def _make_hook():
        try:
            lib = ctypes.CDLL(so_path)
        except OSError:
            return None
        if not hasattr(lib, "axon_start_nrt_profile"):
            return None
        lib.axon_start_nrt_profile.argtypes = [
            ctypes.POINTER(ctypes.c_int64),
            ctypes.c_size_t,
        ]
        lib.axon_start_nrt_profile.restype = ctypes.c_int64
        lib.axon_stop_nrt_profile.argtypes = [ctypes.c_char_p]
        lib.axon_stop_nrt_profile.restype = ctypes.c_int64

        @contextlib.contextmanager
        def _hook(output_dir, device_ids):
            import jax

            jax.devices()
            if device_ids:
                ids = (ctypes.c_int64 * len(device_ids))(*device_ids)
                rc = lib.axon_start_nrt_profile(ids, len(device_ids))
            else:
                rc = lib.axon_start_nrt_profile(None, 0)
            if rc != 0:
                raise RuntimeError(f"axon_start_nrt_profile rc={rc}")
            try:
                yield
            finally:
                n = lib.axon_stop_nrt_profile(str(output_dir).encode())
                print(f"profile: {n} file(s) written to {output_dir}", file=sys.stderr)

        return _hook

    mod = types.ModuleType("antenv.axon_hooks")
    mod.get_axon_ntff_profile_hook = _make_hook
    mod.set_axon_ntff_profile_hook = lambda h: None
    sys.modules["antenv.axon_hooks"] = mod


def _trace_kernel(tc, xT, wq, wk, wv, wo, mask, out):
    import concourse.mybir as mybir
    from concourse.bass import ts

    nc = tc.nc
    f32 = mybir.dt.float32
    bf16 = mybir.dt.bfloat16

    with contextlib.ExitStack() as top:
        # Long-lived pools
        ktr_pool = top.enter_context(tc.tile_pool(name="ktr", bufs=DT))
        ktb_pool = top.enter_context(tc.tile_pool(name="ktb", bufs=DT))
        qt_pool = top.enter_context(tc.tile_pool(name="qt", bufs=DT))
        const_pool = top.enter_context(tc.tile_pool(name="cst", bufs=1))
        ps_pool = top.enter_context(tc.tile_pool(name="ps", bufs=8, space="PSUM"))
        dram_pool = top.enter_context(tc.tile_pool(name="cdram", bufs=4, space="DRAM"))

        # Own-half kT in row layout (filled straight from PSUM evictions);
        # peer-half kT in the same row layout, loaded from kpeer after the
        # ReduceScatter (RS output is just the peer's rows, so both sides of
        # those DMAs use large contiguous per-partition lines).
        ktr = [
            ktr_pool.tile([P, H], bf16, name=f"ktr{i}", tag="ktr") for i in range(DT)
        ]
        ktb = [
            ktb_pool.tile([P, H], bf16, name=f"ktb{i}", tag="ktb") for i in range(DT)
        ]
        qt = [qt_pool.tile([P, H], bf16, name=f"qt{i}", tag="qt") for i in range(DT)]

        mb = const_pool.tile([P, 2], f32, name="mb", tag="mb")
        nc.sync.dma_start(out=mb[:], in_=mask)

        # K-collective staging (2-core groups need Local addr space) and the
        # full-sequence V staging in local DRAM.
        kstage = dram_pool.tile([2, DT, P, H], bf16, name="kstage", tag="kst")
        kpeer = dram_pool.tile([DT, P, H], bf16, name="kpeer", tag="kp")
        vstage = dram_pool.tile([ST, P, D], bf16, name="vstage", tag="vso")

        # ---------------- setup ----------------
        with contextlib.ExitStack() as setup:
            wf_pool = setup.enter_context(tc.tile_pool(name="wf", bufs=2))
            wset_pool = setup.enter_context(tc.tile_pool(name="wset", bufs=2 * DT))
            xf_pool = setup.enter_context(tc.tile_pool(name="xf", bufs=8))
            xb_pool = setup.enter_context(tc.tile_pool(name="xb", bufs=2 * DT))
            esb_pool = setup.enter_context(tc.tile_pool(name="esb", bufs=6))

            def cast_weight(w_ap):
                tiles = []
                for i in range(DT):
                    wf = wf_pool.tile([P, D], f32, name="wf", tag="wf")
                    nc.sync.dma_start(out=wf[:], in_=w_ap[ts(i, P), :])
                    wb = wset_pool.tile([P, D], bf16, name="wb", tag="wset")
                    nc.scalar.copy(wb[:], wf[:])
                    tiles.append(wb)
                return tiles

            def load_x_block(blk):
                xb = []
                for d in range(DT):
                    xf = xf_pool.tile([P, FREE], f32, name="xf", tag="xf")
                    nc.sync.dma_start(out=xf[:], in_=xT[ts(d, P), ts(blk, FREE)])
                    xbt = xb_pool.tile([P, FREE], bf16, name="xbt", tag="xb")
                    nc.scalar.copy(xbt[:], xf[:])
                    xb.append(xbt)
                return xb


            # --- K pass (own half): fill ktr + masked staging for the RS ---
            wkb = cast_weight(wk)
            for blk in range(CH):
                xb = load_x_block(blk)
                for e in range(DT):
                    psum = ps_pool.tile([P, FREE], f32, name="psk", tag="ps")
                    for d in range(DT):
                        nc.tensor.matmul(
                            psum[:],
                            wkb[d][:, ts(e, P)],
                            xb[d][:],
                            start=(d == 0),
                            stop=(d == DT - 1),
                        )
                    nc.vector.tensor_copy(ktr[e][:, ts(blk, FREE)], psum[:])
                    for part in range(2):
                        km = esb_pool.tile([P, FREE], bf16, name="km", tag="esbk")
                        nc.vector.tensor_scalar_mul(
                            km[:], ktr[e][:, ts(blk, FREE)], mb[:, part : part + 1]
                        )
                        # Stores ride the scalar HWDGE queue so they do not
                        # back up the sync queue feeding the x loads.
                        nc.scalar.dma_start(
                            out=kstage[part, e, :, ts(blk, FREE)], in_=km[:]
                        )
            nc.gpsimd.collective_compute(
                "ReduceScatter",
                mybir.AluOpType.add,
                replica_groups=GROUPS,
                ins=[kstage.opt()],
                outs=[kpeer.opt()],
            )

            # --- combined V (full sequence) + Q (own half) pass ---
            wvb = cast_weight(wv)
            wqb = cast_weight(wq)
            for blk in range(SBLK):
                xb = load_x_block(blk)
                for ss in range(FREE // P):
                    s_tile = blk * (FREE // P) + ss
                    vt = esb_pool.tile([P, D], bf16, name="vt", tag="esbv")
                    for nh in range(2):
                        psum = ps_pool.tile([P, FREE], f32, name="psv", tag="ps")
                        for d in range(DT):
                            nc.tensor.matmul(
                                psum[:],
                                xb[d][:, ts(ss, P)],
                                wvb[d][:, ts(nh, FREE)],
                                start=(d == 0),
                                stop=(d == DT - 1),
                            )
                        nc.vector.tensor_copy(vt[:, ts(nh, FREE)], psum[:])
                    nc.scalar.dma_start(out=vstage[s_tile], in_=vt[:])
                if blk < CH:  # q projection for the own half
                    for e in range(DT):
                        psum = ps_pool.tile([P, FREE], f32, name="psq", tag="ps")
                        for d in range(DT):
                            nc.tensor.matmul(
                                psum[:],
                                wqb[d][:, ts(e, P)],
                                xb[d][:],
                                start=(d == 0),
                                stop=(d == DT - 1),
                            )
                        nc.vector.tensor_copy(qt[e][:, ts(blk, FREE)], psum[:])

        # Peer-half kT into SBUF (waits on the K ReduceScatter via tile
        # deps). Issued from the otherwise-idle SWDGE queue so the wait on
        # the collective cannot stall the sync/scalar DMA queues.
        for e in range(DT):
            nc.gpsimd.dma_start(out=ktb[e][:], in_=kpeer[e])

        # w_o cast (after setup pools release)
        wo_pool = top.enter_context(tc.tile_pool(name="wob", bufs=DT))
        wof_pool = top.enter_context(tc.tile_pool(name="wof", bufs=2))
        wob = []
        for i in range(DT):
            wf = wof_pool.tile([P, D], f32, name="wof", tag="wof")
            nc.sync.dma_start(out=wf[:], in_=wo[ts(i, P), :])
            wb = wo_pool.tile([P, D], bf16, name="wob", tag="wob")
            nc.scalar.copy(wb[:], wf[:])
            wob.append(wb)

        # ---------------- main loop over t-chunks ----------------
        sct_pool = top.enter_context(tc.tile_pool(name="sct", bufs=ST))
        att_pool = top.enter_context(tc.tile_pool(name="att", bufs=2 * DT))
        vld_pool = top.enter_context(tc.tile_pool(name="vld", bufs=6))
        ost_pool = top.enter_context(tc.tile_pool(name="ost", bufs=4))

        for c in range(CH):
            # scores^T [s, t-chunk]: own half from ktr, peer half from ktb
            sct = []
            for st in range(ST):
                psum = ps_pool.tile([P, FREE], f32, name="pss", tag="ps")
                for e in range(DT):
                    lhsT = (
                        ktr[e][:, ts(st, P)]
                        if st < STH
                        else ktb[e][:, ts(st - STH, P)]
                    )
                    nc.tensor.matmul(
                        psum[:],
                        lhsT,
                        qt[e][:, ts(c, FREE)],
                        start=(e == 0),
                        stop=(e == DT - 1),
                    )
                sc = sct_pool.tile([P, FREE], bf16, name="sc", tag="sct")
                nc.vector.tensor_copy(sc[:], psum[:])
                sct.append(sc)

            # attn^T [e, t-chunk]: all 8 PSUM banks accumulate over s, so v
            # streams through SBUF exactly once per chunk.
            att = [None] * DT
            accs = [
                ps_pool.tile([P, FREE], f32, name=f"acc{j}", tag="ps")
                for j in range(DT)
            ]
            for st in range(ST):
                vt = vld_pool.tile([P, D], bf16, name="vl", tag="vld")
                nc.sync.dma_start(out=vt[:], in_=vstage[st])
                for e in range(DT):
                    nc.tensor.matmul(
                        accs[e][:],
                        vt[:, ts(e, P)],
                        sct[st][:],
                        start=(st == 0),
                        stop=(st == ST - 1),
                    )
            for e in range(DT):
                a = att_pool.tile([P, FREE], bf16, name="at", tag="att")
                nc.vector.tensor_copy(a[:], accs[e][:])
                att[e] = a

            # output projection [t-chunk, 1024]
            for tt in range(FREE // P):
                for nh in range(2):
                    psum = ps_pool.tile([P, FREE], f32, name="pso", tag="ps")
                    for e in range(DT):
                        nc.tensor.matmul(
                            psum[:],
                            att[e][:, ts(tt, P)],
                            wob[e][:, ts(nh, FREE)],
                            start=(e == 0),
                            stop=(e == DT - 1),
                        )
                    ot = ost_pool.tile([P, FREE], f32, name="ot", tag="ost")
                    nc.scalar.copy(ot[:], psum[:])
                    row = c * FREE + tt * P
                    nc.scalar.dma_start(
                        out=out[row : row + P, ts(nh, FREE)], in_=ot[:]
                    )


def _build():
    _install_axon_ntff_shim()
    import concourse.mybir as mybir
    import concourse.tile as tile
    from concourse import bacc

    f32 = mybir.dt.float32
    nc = bacc.Bacc("TRN2", target_bir_lowering=False, debug=False, num_devices=NCORES)
    xT = nc.dram_tensor("xT", [D, T], f32, kind="ExternalInput").ap()
    wq = nc.dram_tensor("wq", [D, D], f32, kind="ExternalInput").ap()
    wk = nc.dram_tensor("wk", [D, D], f32, kind="ExternalInput").ap()
    wv = nc.dram_tensor("wv", [D, D], f32, kind="ExternalInput").ap()
    wo = nc.dram_tensor("wo", [D, D], f32, kind="ExternalInput").ap()
    mask = nc.dram_tensor("mask", [P, 2], f32, kind="ExternalInput").ap()
    out = nc.dram_tensor("out", [H, D], f32, kind="ExternalOutput").ap()

    with tile.TileContext(nc) as tc:
        _trace_kernel(tc, xT, wq, wk, wv, wo, mask, out)
    nc.compile()
    return nc


def kernel(x, w_q, w_k, w_v, w_o):
    global LAST_RESULTS
    from concourse import bass_utils

    if "nc" not in _STATE:
        _STATE["nc"] = _build()
    nc = _STATE["nc"]

    x = np.ascontiguousarray(x, dtype=np.float32)
    in_maps = []
    for core in range(NCORES):
        b, half = core // 2, core % 2
        own = x[b, half * H : (half + 1) * H]
        oth = x[b, (1 - half) * H : (2 - half) * H]
        xT = np.ascontiguousarray(np.concatenate([own, oth], axis=0).T)
        m = np.zeros((P, 2), dtype=np.float32)
        m[:, 1 - half] = 1.0  # zero own slot; pair position == half
        in_maps.append(
            {
                "xT": xT,
                "wq": np.ascontiguousarray(w_q, dtype=np.float32),
                "wk": np.ascontiguousarray(w_k, dtype=np.float32),
                "wv": np.ascontiguousarray(w_v, dtype=np.float32),
                "wo": np.ascontiguousarray(w_o, dtype=np.float32),
                "mask": m,
            }
        )

    LAST_RESULTS = bass_utils.run_bass_kernel_spmd(
        nc, in_maps, core_ids=list(range(NCORES))
    )
    out = np.empty((B, T, D), dtype=np.float32)
    for core in range(NCORES):
        b, half = core // 2, core % 2
        out[b, half * H : (half + 1) * H] = LAST_RESULTS.results[core]["out"]
    return out



# revision 5
# speedup vs baseline: 3.3830x; 3.3830x over previous
"""Trainium2 Bass kernel: unnormalized single-head attention block.

Computes, for x [4, 4096, 1024] and w_q/w_k/w_v/w_o [1024, 1024] (all fp32):
    q = x @ w_q ; k = x @ w_k ; v = x @ w_v
    scores = q @ k.T            (no softmax)
    out = (scores @ v) @ w_o

Because there is no softmax, the chain is associative and collapses to
    out_b = x_b @ [ w_q @ w_k.T @ (x_b.T @ x_b) @ w_v @ w_o ]
which replaces the two T x T matmuls (34 GFLOP each per batch) with a
Gram matrix G_b = x_b.T @ x_b and a short chain of 1024^3 matmuls:
~90 GFLOP total instead of ~412 GFLOP.

Sharding: 8 NeuronCores = (4 batches) x (2 sequence halves). Each core
computes G over its own 2048-row half; the pair's halves are summed with a
2 MB bf16 AllReduce over groups [[0,1],[2,3],[4,5],[6,7]] (G_b = own + peer).
While the collective is in flight the PE computes the batch-independent
products AT = w_k @ w_q.T and C = w_v @ w_o, so the tensor engine never
idles. Afterwards R = G @ C (G is symmetric, so G serves as its own lhsT),
M = AT.T @ R, and out rows for the own half: out = x_own @ M.

Device math is bf16 with fp32 PSUM accumulation. The host ships bf16
tensors directly (x half in both natural and transposed layout; w_q/w_k/w_v
transposed) so no on-device transposes or casts are needed.
"""

import contextlib
import ctypes
import os
import sys
import types

import numpy as np

B = 4
T = 4096
D = 1024
H = T // 2          # rows per core
P = 128             # SBUF partitions
NCORES = 8
DT = D // P         # 8 tiles along any 1024 dim
TT = H // P         # 16 own-half t-tiles
FREE = 512          # matmul moving free dim / PSUM bank width (fp32)
KC = D // FREE      # 2 free-dim chunks of 512 along a 1024 dim
GROUPS = [[0, 1], [2, 3], [4, 5], [6, 7]]

_STATE = {}
LAST_RESULTS = None


def _install_axon_ntff_shim():
    """bass_utils(trace=True) under axon imports antenv.axon_hooks, which the
    agent image lacks. Provide the documented ctypes equivalent so tracing
    works; degrades to hook=None when the .so has no profile symbols."""
    try:
        import antenv.axon_hooks  # noqa: F401
        return
    except ImportError:
        pass

    so_path = "/opt/axon/libaxon_pjrt.so"

    def _make_hook():
        try:
            lib = ctypes.CDLL(so_path)
        except OSError:
            return None
        if not hasattr(lib, "axon_start_nrt_profile"):
            return None
        lib.axon_start_nrt_profile.argtypes = [
            ctypes.POINTER(ctypes.c_int64),
            ctypes.c_size_t,
        ]
        lib.axon_start_nrt_profile.restype = ctypes.c_int64
        lib.axon_stop_nrt_profile.argtypes = [ctypes.c_char_p]
        lib.axon_stop_nrt_profile.restype = ctypes.c_int64

        @contextlib.contextmanager
        def _hook(output_dir, device_ids):
            import jax

            jax.devices()
            if device_ids:
                ids = (ctypes.c_int64 * len(device_ids))(*device_ids)
                rc = lib.axon_start_nrt_profile(ids, len(device_ids))
            else:
                rc = lib.axon_start_nrt_profile(None, 0)
            if rc != 0:
                raise RuntimeError(f"axon_start_nrt_profile rc={rc}")
            try:
                yield
            finally:
                n = lib.axon_stop_nrt_profile(str(output_dir).encode())
                print(f"profile: {n} file(s) written to {output_dir}", file=sys.stderr)

        return _hook

    mod = types.ModuleType("antenv.axon_hooks")
    mod.get_axon_ntff_profile_hook = _make_hook
    mod.set_axon_ntff_profile_hook = lambda h: None
    sys.modules["antenv.axon_hooks"] = mod


def _trace_kernel(tc, xn, xt, wqT, wkT, wvT, wo, out):
    import concourse.mybir as mybir
    from concourse.bass import ts

    nc = tc.nc
    f32 = mybir.dt.float32
    bf16 = mybir.dt.bfloat16

    with contextlib.ExitStack() as top:
        ps_pool = top.enter_context(tc.tile_pool(name="ps", bufs=8, space="PSUM"))
        dram_pool = top.enter_context(tc.tile_pool(name="cdram", bufs=2, space="DRAM"))
        at_pool = top.enter_context(tc.tile_pool(name="at", bufs=DT))
        c_pool = top.enter_context(tc.tile_pool(name="c", bufs=DT))

        # Collective staging in local DRAM (pair groups need Local addr space).
        gsrc = dram_pool.tile([DT, P, D], bf16, name="gsrc", tag="gsrc")
        gsum = dram_pool.tile([DT, P, D], bf16, name="gsum", tag="gsum")

        with contextlib.ExitStack() as setup:
            xn_pool = setup.enter_context(tc.tile_pool(name="xn", bufs=TT))
            w_pool = setup.enter_context(tc.tile_pool(name="w", bufs=4 * DT))
            gown_pool = setup.enter_context(tc.tile_pool(name="gown", bufs=DT))

            xns = []
            for t in range(TT):
                xv = xn_pool.tile([P, D], bf16, name=f"xn{t}", tag="xn")
                nc.sync.dma_start(out=xv[:], in_=xn[ts(t, P), :])
                xns.append(xv)

            def load_w(w_ap, tag):
                tiles = []
                for i in range(DT):
                    wt = w_pool.tile([P, D], bf16, name=f"{tag}{i}", tag="w")
                    nc.sync.dma_start(out=wt[:], in_=w_ap[ts(i, P), :])
                    tiles.append(wt)
                return tiles

            wk_t = load_w(wkT, "wk")
            wq_t = load_w(wqT, "wq")
            wv_t = load_w(wvT, "wv")
            wo_t = load_w(wo, "wo")

            # --- own-half Gram matrix G[j,k] = sum_t x[t,j] x[t,k] ---
            gown = [
                gown_pool.tile([P, D], bf16, name=f"go{j}", tag="gown")
                for j in range(DT)
            ]
            for jt in range(DT):
                for kc in range(KC):
                    psum = ps_pool.tile([P, FREE], f32, name="psg", tag="ps")
                    for t in range(TT):
                        nc.tensor.matmul(
                            psum[:],
                            xns[t][:, ts(jt, P)],
                            xns[t][:, ts(kc, FREE)],
                            start=(t == 0),
                            stop=(t == TT - 1),
                        )
                    nc.vector.tensor_copy(gown[jt][:, ts(kc, FREE)], psum[:])
                nc.scalar.dma_start(out=gsrc[jt], in_=gown[jt][:])

            # Pairwise sum: each core receives G_own + G_peer = full-batch G.
            nc.gpsimd.collective_compute(
                "AllReduce",
                mybir.AluOpType.add,
                replica_groups=GROUPS,
                ins=[gsrc.opt()],
                outs=[gsum.opt()],
            )

            # --- batch-independent products, overlapped with the collective ---
            # AT[j,d] = (w_q @ w_k.T).T = sum_i wk[j,i] wq[d,i]
            ats = [
                at_pool.tile([P, D], bf16, name=f"at{j}", tag="at") for j in range(DT)
            ]
            for jt in range(DT):
                for dc in range(KC):
                    psum = ps_pool.tile([P, FREE], f32, name="psa", tag="ps")
                    for i in range(DT):
                        nc.tensor.matmul(
                            psum[:],
                            wk_t[i][:, ts(jt, P)],
                            wq_t[i][:, ts(dc, FREE)],
                            start=(i == 0),
                            stop=(i == DT - 1),
                        )
                    nc.vector.tensor_copy(ats[jt][:, ts(dc, FREE)], psum[:])

            # C[k,e] = (w_v @ w_o)[k,e] = sum_l wv[k,l] wo[l,e]
            cs = [c_pool.tile([P, D], bf16, name=f"c{k}", tag="c") for k in range(DT)]
            for kt in range(DT):
                for ec in range(KC):
                    psum = ps_pool.tile([P, FREE], f32, name="psc", tag="ps")
                    for l in range(DT):
                        nc.tensor.matmul(
                            psum[:],
                            wv_t[l][:, ts(kt, P)],
                            wo_t[l][:, ts(ec, FREE)],
                            start=(l == 0),
                            stop=(l == DT - 1),
                        )
                    nc.vector.tensor_copy(cs[kt][:, ts(ec, FREE)], psum[:])

        # Late-phase pools, created after the setup pools release their SBUF.
        xt_pool = top.enter_context(tc.tile_pool(name="xt", bufs=DT))
        gf_pool = top.enter_context(tc.tile_pool(name="gf", bufs=DT))
        r_pool = top.enter_context(tc.tile_pool(name="r", bufs=DT))
        m_pool = top.enter_context(tc.tile_pool(name="m", bufs=DT))
        ot_pool = top.enter_context(tc.tile_pool(name="ot", bufs=4))

        # x.T tiles for the final out = x @ M matmul.
        xts = []
        for i in range(DT):
            xv = xt_pool.tile([P, H], bf16, name=f"xt{i}", tag="xt")
            nc.sync.dma_start(out=xv[:], in_=xt[ts(i, P), :])
            xts.append(xv)

        # Full G into SBUF (waits on the AllReduce via tile deps; rides the
        # otherwise-idle SWDGE queue so the wait cannot stall the load queues).
        gfs = []
        for kt in range(DT):
            gf = gf_pool.tile([P, D], bf16, name=f"gf{kt}", tag="gf")
            nc.gpsimd.dma_start(out=gf[:], in_=gsum[kt])
            gfs.append(gf)

        # R[j,e] = (G @ C)[j,e]; G is symmetric so its tiles serve as lhsT.
        rs = []
        for jt in range(DT):
            rt = r_pool.tile([P, D], bf16, name=f"r{jt}", tag="r")
            for ec in range(KC):
                psum = ps_pool.tile([P, FREE], f32, name="psr", tag="ps")
                for kt in range(DT):
                    nc.tensor.matmul(
                        psum[:],
                        gfs[kt][:, ts(jt, P)],
                        cs[kt][:, ts(ec, FREE)],
                        start=(kt == 0),
                        stop=(kt == DT - 1),
                    )
                nc.vector.tensor_copy(rt[:, ts(ec, FREE)], psum[:])
            rs.append(rt)

        # M[d,e] = (w_q @ w_k.T @ R)[d,e] = sum_j AT[j,d] R[j,e]
        ms = []
        for dt_ in range(DT):
            mt = m_pool.tile([P, D], bf16, name=f"m{dt_}", tag="m")
            for ec in range(KC):
                psum = ps_pool.tile([P, FREE], f32, name="psm", tag="ps")
                for jt in range(DT):
                    nc.tensor.matmul(
                        psum[:],
                        ats[jt][:, ts(dt_, P)],
                        rs[jt][:, ts(ec, FREE)],
                        start=(jt == 0),
                        stop=(jt == DT - 1),
                    )
                nc.vector.tensor_copy(mt[:, ts(ec, FREE)], psum[:])
            ms.append(mt)

        # out[t,e] = sum_d x[t,d] M[d,e], own-half rows.
        for tt in range(TT):
            for ec in range(KC):
                psum = ps_pool.tile([P, FREE], f32, name="pso", tag="ps")
                for dt_ in range(DT):
                    nc.tensor.matmul(
                        psum[:],
                        xts[dt_][:, ts(tt, P)],
                        ms[dt_][:, ts(ec, FREE)],
                        start=(dt_ == 0),
                        stop=(dt_ == DT - 1),
                    )
                o = ot_pool.tile([P, FREE], f32, name="ot", tag="ot")
                nc.scalar.copy(o[:], psum[:])
                nc.scalar.dma_start(out=out[ts(tt, P), ts(ec, FREE)], in_=o[:])


def _build():
    _install_axon_ntff_shim()
    import concourse.mybir as mybir
    import concourse.tile as tile
    from concourse import bacc

    f32 = mybir.dt.float32
    bf16 = mybir.dt.bfloat16
    nc = bacc.Bacc("TRN2", target_bir_lowering=False, debug=False, num_devices=NCORES)
    xn = nc.dram_tensor("xn", [H, D], bf16, kind="ExternalInput").ap()
    xt = nc.dram_tensor("xt", [D, H], bf16, kind="ExternalInput").ap()
    wqT = nc.dram_tensor("wqT", [D, D], bf16, kind="ExternalInput").ap()
    wkT = nc.dram_tensor("wkT", [D, D], bf16, kind="ExternalInput").ap()
    wvT = nc.dram_tensor("wvT", [D, D], bf16, kind="ExternalInput").ap()
    wo = nc.dram_tensor("wo", [D, D], bf16, kind="ExternalInput").ap()
    out = nc.dram_tensor("out", [H, D], f32, kind="ExternalOutput").ap()

    with tile.TileContext(nc) as tc:
        _trace_kernel(tc, xn, xt, wqT, wkT, wvT, wo, out)
    nc.compile()
    return nc


def kernel(x, w_q, w_k, w_v, w_o):
    global LAST_RESULTS
    import ml_dtypes
    from concourse import bass_utils

    if "nc" not in _STATE:
        _STATE["nc"] = _build()
    nc = _STATE["nc"]

    bf16 = ml_dtypes.bfloat16
    x = np.ascontiguousarray(x, dtype=np.float32)
    wqT = np.asarray(w_q, dtype=np.float32).T.astype(bf16)
    wkT = np.asarray(w_k, dtype=np.float32).T.astype(bf16)
    wvT = np.asarray(w_v, dtype=np.float32).T.astype(bf16)
    wob = np.ascontiguousarray(np.asarray(w_o, dtype=np.float32)).astype(bf16)

    in_maps = []
    for core in range(NCORES):
        b, half = core // 2, core % 2
        xh = x[b, half * H : (half + 1) * H]
        in_maps.append(
            {
                "xn": xh.astype(bf16),
                "xt": xh.T.astype(bf16),
                "wqT": wqT,
                "wkT": wkT,
                "wvT": wvT,
                "wo": wob,
            }
        )

    LAST_RESULTS = bass_utils.run_bass_kernel_spmd(
        nc, in_maps, core_ids=list(range(NCORES))
    )
    out = np.empty((B, T, D), dtype=np.float32)
    for core in range(NCORES):
        b, half = core // 2, core % 2
        out[b, half * H : (half + 1) * H] = LAST_RESULTS.results[core]["out"]
    return out
